# revision 8
# baseline (speedup 1.0000x reference)
"""Multi-head attention block (q/k/v projections + softmax attention +
out-projection) distributed over 8 TRN2 NeuronCores.

Sharding: core c handles batch b = c//2 and query rows [h*1024, (h+1)*1024),
h = c%2. Each core keeps the full kv of its batch (kv projections are
recomputed per query-half) so no inter-core collective is needed; the full
output is assembled host-side from disjoint shards.

Per-core dataflow:
  q/kv -> bf16 DRAM scratch (SWDGE cast-DMA, per-128-column slice)
       -> qT/kvT [model_dim, seq] in SBUF via HWDGE DMA-transpose
  projections (bf16 matmuls, fp32 PSUM):
       qhT/khT [inner, seq] transposed layout (stored fp32r for the score
       matmuls), vh [seq_k, head*(64+1)] natural layout bf16 with a ones
       column (P@[V|1] then yields the softmax denominator for free)
  attention per head-pair (two heads row-packed in the PE via tile_position,
  contraction dim HEAD_DIM=64):
       scores S^T[k, q] on PSUM (fp32r) -> exp(s/8) fused on ScalarE -> bf16
       -> PV [65, q] PSUM accumulation over the 16 k tiles (bf16 operands)
       -> denominator row -> Kc=1 ones-matmul broadcast -> fast reciprocal
       -> multiply -> attnT (fp32r)
  out-projection (fp32r) + broadcast bias.

k/q projections for later head-pairs are interleaved into the attention
k-loops as PE filler, and each block's normalization is deferred until the
next block's first scores/exp have been issued, so neither the TensorEngine
nor the ScalarEngine stalls at block boundaries.
"""

import sys

sys.path.insert(0, "/opt/trn_rl_repo")

import numpy as np

B, NQ_FULL, NK = 4, 2048, 2048
NQ = 1024          # per-core query rows
DQ, DKV = 512, 768
HEADS, DH = 8, 64
INNER = 512
DA = DH + 1        # head dim + ones column
N_CORES = 8

_cache = {}


def _build():
    import concourse.bass as bass
    import concourse.tile as tile
    from concourse import bacc, mybir

    F32 = mybir.dt.float32
    F32R = mybir.dt.float32r
    BF16 = mybir.dt.bfloat16
    EXP = mybir.ActivationFunctionType.Exp

    nc = bacc.Bacc("TRN2", target_bir_lowering=False, debug=False,
                   enable_asserts=True, num_devices=N_CORES)

    q_d = nc.dram_tensor("q", [NQ, DQ], F32, kind="ExternalInput").ap()
    kv_d = nc.dram_tensor("kv", [NK, DKV], F32, kind="ExternalInput").ap()
    wq_d = nc.dram_tensor("Wq", [DQ, INNER], F32, kind="ExternalInput").ap()
    wk_d = nc.dram_tensor("Wk", [DKV, INNER], F32, kind="ExternalInput").ap()
    wv_d = nc.dram_tensor("Wv", [DKV, INNER], F32, kind="ExternalInput").ap()
    wo_d = nc.dram_tensor("Wo", [INNER, DQ], F32, kind="ExternalInput").ap()
    bo_d = nc.dram_tensor("bo", [DQ], F32, kind="ExternalInput").ap()
    out_d = nc.dram_tensor("out", [NQ, DQ], F32, kind="ExternalOutput").ap()

    MT_Q = DQ // 128      # 4
    MT_KV = DKV // 128    # 6
    IT = INNER // 128     # 4 inner tiles (= head pairs)
    KT = NK // 128        # 16
    QB = NQ // 512        # 2
    PAIRS = HEADS // 2    # 4

    with tile.TileContext(nc) as tc:
        with (
            tc.tile_pool(name="consts", bufs=1) as consts,
            tc.tile_pool(name="wpool", bufs=1) as wpool,
            tc.tile_pool(name="xT", bufs=1) as xT_pool,
            tc.tile_pool(name="proj", bufs=1) as proj_pool,
            tc.tile_pool(name="attnT", bufs=1) as attnT_pool,
            tc.tile_pool(name="exps", bufs=5) as exps_pool,
            tc.tile_pool(name="outs", bufs=2) as outs_pool,
            tc.tile_pool(name="dram", bufs=1, space="DRAM") as dram_pool,
            tc.tile_pool(name="mm", bufs=2, space="PSUM") as ps_mm,
            tc.tile_pool(name="sc", bufs=2, space="PSUM") as ps_sc,
            tc.tile_pool(name="pv", bufs=2, space="PSUM") as ps_pv,
        ):
            # ---- bf16 scratch in DRAM (SWDGE cast), then DMA-transpose.
            # kv first: it gates khT/vh and thus the whole attention pipe.
            kv_bf = dram_pool.tile([NK, DKV], BF16, tag="kv_bf")
            q_bf = dram_pool.tile([NQ, DQ], BF16, tag="q_bf")
            kvT = [xT_pool.tile([128, NK], BF16, tag=f"kvT{mt}", name=f"kvT{mt}")
                   for mt in range(MT_KV)]
            qT = [xT_pool.tile([128, NQ], BF16, tag=f"qT{mt}", name=f"qT{mt}")
                  for mt in range(MT_Q)]
            for mt in range(MT_KV):
                cs = slice(mt * 128, (mt + 1) * 128)
                nc.gpsimd.dma_start(out=kv_bf[:, cs], in_=kv_d[:, cs])
                nc.sync.dma_start_transpose(out=kvT[mt][:], in_=kv_bf[:, cs])
            for mt in range(MT_Q):
                cs = slice(mt * 128, (mt + 1) * 128)
                nc.gpsimd.dma_start(out=q_bf[:, cs], in_=q_d[:, cs])
                nc.sync.dma_start_transpose(out=qT[mt][:], in_=q_bf[:, cs])

            # ---- weights (HWDGE loads queue behind the transposes) ----
            wk_b = wpool.tile([128, MT_KV, 512], BF16, tag="wk")
            wq_b = wpool.tile([128, MT_Q, 512], BF16, tag="wq")
            wv_b = wpool.tile([128, MT_KV, 512], BF16, tag="wv")
            wo_r = wpool.tile([128, IT, 512], F32R, tag="wo")
            with tc.tile_pool(name="wstage", bufs=2) as wstage:
                for wd, wt, mt in ((wk_d, wk_b, MT_KV), (wq_d, wq_b, MT_Q),
                                   (wv_d, wv_b, MT_KV), (wo_d, wo_r, IT)):
                    st = wstage.tile([128, mt, 512], F32, tag="wst")
                    nc.sync.dma_start(st[:], wd.rearrange("(t p) i -> p t i", p=128))
                    nc.vector.tensor_copy(wt[:], st[:])

            # ---- constants ----
            ones1f = consts.tile([1, 64], F32)
            nc.vector.memset(ones1f[:], 1.0)
            ones1 = consts.tile([1, 64], F32R)
            nc.vector.tensor_copy(ones1[:], ones1f[:])
            ones8 = consts.tile([128, 8, 1], BF16)
            ones8f = consts.tile([128, 8, 1], F32)
            nc.vector.memset(ones8f[:], 1.0)
            nc.vector.tensor_copy(ones8[:], ones8f[:])
            bo_b = consts.tile([128, DQ], F32)
            nc.gpsimd.dma_start(
                out=bo_b[:],
                in_=bass.AP(tensor=bo_d.tensor, offset=bo_d.offset,
                            ap=[[0, 128]] + list(bo_d.ap)),
            )

            # ---- projection outputs ----
            qhT = [proj_pool.tile([128, NQ], F32R, tag=f"qhT{i}", name=f"qhT{i}")
                   for i in range(IT)]
            khT = [proj_pool.tile([128, NK], F32R, tag=f"khT{i}", name=f"khT{i}")
                   for i in range(IT)]
            vh = [proj_pool.tile([128, HEADS, DA], BF16, tag=f"vh{k}", name=f"vh{k}")
                  for k in range(KT)]
            attnT = [attnT_pool.tile([128, NQ], F32R, tag=f"at{i}", name=f"at{i}")
                     for i in range(IT)]

            def emit_khT(it, nb):
                pp = ps_mm.tile([128, 512], F32, tag="mm", name="pp")
                for mt in range(MT_KV):
                    nc.tensor.matmul(
                        pp[:], wk_b[:, mt, it * 128:(it + 1) * 128],
                        kvT[mt][:, nb * 512:(nb + 1) * 512],
                        start=(mt == 0), stop=(mt == MT_KV - 1))
                nc.vector.tensor_copy(khT[it][:, nb * 512:(nb + 1) * 512], pp[:])

            def emit_qhT(it, nb):
                pp = ps_mm.tile([128, 512], F32, tag="mm", name="pp")
                for mt in range(MT_Q):
                    nc.tensor.matmul(
                        pp[:], wq_b[:, mt, it * 128:(it + 1) * 128],
                        qT[mt][:, nb * 512:(nb + 1) * 512],
                        start=(mt == 0), stop=(mt == MT_Q - 1))
                nc.vector.tensor_copy(qhT[it][:, nb * 512:(nb + 1) * 512], pp[:])

            def emit_vh(kt):
                pp = ps_mm.tile([128, 512], F32, tag="mm", name="pp")
                for mt in range(MT_KV):
                    nc.tensor.matmul(
                        pp[:], kvT[mt][:, kt * 128:(kt + 1) * 128],
                        wv_b[:, mt, :],
                        start=(mt == 0), stop=(mt == MT_KV - 1))
                nc.vector.tensor_copy(
                    vh[kt][:, :, 0:DH],
                    pp[:].rearrange("p (h d) -> p h d", h=HEADS))
                nc.vector.tensor_copy(vh[kt][:, :, DH:DA], ones8[:])

            # pre-attention minimum: pair-0 projections and the first v tiles
            for nb in range(NK // 512):
                emit_khT(0, nb)
            for nb in range(QB):
                emit_qhT(0, nb)
            emit_vh(0)
            emit_vh(1)

            # PE filler work interleaved into each pair's attention k-loops
            fillers = {t: [] for t in range(PAIRS)}
            fillers[0] = ([(lambda kt=kt: emit_vh(kt)) for kt in range(2, KT)]
                          + [(lambda nb=nb: emit_khT(1, nb)) for nb in range(NK // 512)]
                          + [(lambda nb=nb: emit_qhT(1, nb)) for nb in range(QB)])
            for t in (1, 2):
                fillers[t] = ([(lambda nb=nb, it=t + 1: emit_khT(it, nb))
                               for nb in range(NK // 512)]
                              + [(lambda nb=nb, it=t + 1: emit_qhT(it, nb))
                                 for nb in range(QB)])
            fillers[3] = []

            # ---- attention ----
            pending_norm = [None]

            def make_norm(t, qb, pvA, pvB):
                def emit():
                    qs = slice(qb * 512, (qb + 1) * 512)
                    dsb = exps_pool.tile([1, 1024], F32R, tag="exp", name="dsb")
                    nc.vector.tensor_copy(dsb[0:1, 0:512], pvA[DH:DA, :])
                    nc.vector.tensor_copy(dsb[0:1, 512:1024], pvB[DH:DA, :])
                    dba = ps_mm.tile([64, 512], F32, tag="mm", name="dba")
                    dbb = ps_mm.tile([64, 512], F32, tag="mm", name="dbb")
                    nc.tensor.matmul(dba[:], ones1[:], dsb[0:1, 0:512],
                                     start=True, stop=True)
                    nc.tensor.matmul(dbb[:], ones1[:], dsb[0:1, 512:1024],
                                     start=True, stop=True)
                    rb = exps_pool.tile([64, 1024], F32, tag="exp", name="rb")
                    nc.vector.reciprocal_approx_fast(rb[:, 0:512], dba[:])
                    nc.vector.reciprocal_approx_fast(rb[:, 512:1024], dbb[:])
                    nc.vector.tensor_mul(attnT[t][0:64, qs],
                                         pvA[0:DH, :], rb[:, 0:512])
                    nc.vector.tensor_mul(attnT[t][64:128, qs],
                                         pvB[0:DH, :], rb[:, 512:1024])
                return emit

            for t in range(PAIRS):
                hA, hB = 2 * t, 2 * t + 1
                todo = fillers[t]
                fi = 0
                for qb in range(QB):
                    qs = slice(qb * 512, (qb + 1) * 512)

                    def emit_scores(kt):
                        ks = slice(kt * 128, (kt + 1) * 128)
                        sc = ps_sc.tile([128, 1024], F32, tag="sc", name="sc")
                        nc.tensor.matmul(
                            sc[:, 0:512],
                            khT[t][0:64, ks], qhT[t][0:64, qs],
                            start=True, stop=True, tile_position=(0, 0))
                        nc.tensor.matmul(
                            sc[:, 512:1024],
                            khT[t][64:128, ks], qhT[t][64:128, qs],
                            start=True, stop=True, tile_position=(64, 0))
                        ex = exps_pool.tile([128, 1024], BF16, tag="exp", name="ex")
                        nc.scalar.activation(ex[:], sc[:], EXP,
                                             scale=float(DH) ** -0.5)
                        return ex

                    def emit_pv(kt, ex, pvA, pvB):
                        nc.tensor.matmul(pvA[:], vh[kt][:, hA, :], ex[:, 0:512],
                                         start=(kt == 0), stop=(kt == KT - 1))
                        nc.tensor.matmul(pvB[:], vh[kt][:, hB, :], ex[:, 512:1024],
                                         start=(kt == 0), stop=(kt == KT - 1))

                    # two scores/exp blocks up-front (they need no pv slot) so
                    # ScalarE is fed across the previous block's normalization
                    pre = [emit_scores(kt) for kt in range(2)]
                    if pending_norm[0] is not None:
                        pending_norm[0]()   # frees the previous pv tiles
                        pending_norm[0] = None
                    pvA = ps_pv.tile([DA, 512], F32, tag="pv", name="pvA")
                    pvB = ps_pv.tile([DA, 512], F32, tag="pv", name="pvB")
                    for kt, ex in enumerate(pre):
                        emit_pv(kt, ex, pvA, pvB)
                        if fi < len(todo):
                            todo[fi]()
                            fi += 1
                    for kt in range(2, KT):
                        ex = emit_scores(kt)
                        emit_pv(kt, ex, pvA, pvB)
                        if fi < len(todo):
                            todo[fi]()
                            fi += 1
                    pending_norm[0] = make_norm(t, qb, pvA, pvB)
                while fi < len(todo):
                    todo[fi]()
                    fi += 1
            pending_norm[0]()

            # ---- output projection + bias ----
            for nt in range(NQ // 128):
                ns = slice(nt * 128, (nt + 1) * 128)
                po = ps_mm.tile([128, 512], F32, tag="mm", name="po")
                for it in range(IT):
                    nc.tensor.matmul(po[:], attnT[it][:, ns], wo_r[:, it, :],
                                     start=(it == 0), stop=(it == IT - 1))
                ot = outs_pool.tile([128, DQ], F32, tag="ot", name="ot")
                nc.vector.tensor_add(ot[:], po[:], bo_b[:])
                nc.sync.dma_start(out_d[ns, :], ot[:])

    nc.compile()
    return nc


def kernel(q, kv, Wq, Wk, Wv, Wo, bo):
    from concourse.bass_utils import run_bass_kernel_spmd

    q = np.asarray(q, dtype=np.float32)
    kv = np.asarray(kv, dtype=np.float32)
    Wq = np.ascontiguousarray(np.asarray(Wq, dtype=np.float32))
    Wk = np.ascontiguousarray(np.asarray(Wk, dtype=np.float32))
    Wv = np.ascontiguousarray(np.asarray(Wv, dtype=np.float32))
    Wo = np.ascontiguousarray(np.asarray(Wo, dtype=np.float32))
    bo = np.ascontiguousarray(np.asarray(bo, dtype=np.float32))

    if "nc" not in _cache:
        _cache["nc"] = _build()
    nc = _cache["nc"]

    in_maps = []
    for c in range(N_CORES):
        b, h = c // 2, c % 2
        in_maps.append({
            "q": np.ascontiguousarray(q[b, h * NQ:(h + 1) * NQ]),
            "kv": np.ascontiguousarray(kv[b]),
            "Wq": Wq, "Wk": Wk, "Wv": Wv, "Wo": Wo, "bo": bo,
        })
    res = run_bass_kernel_spmd(nc, in_maps, core_ids=list(range(N_CORES)))
    out = np.empty((B, NQ_FULL, DQ), dtype=np.float32)
    for c in range(N_CORES):
        b, h = c // 2, c % 2
        out[b, h * NQ:(h + 1) * NQ] = res.results[c]["out"]
    return out
